# revision 31
# baseline (speedup 1.0000x reference)
"""Trainium2 Bass kernel for nn_MultiHeadReadOutAttention.

Computation (per (b,s), reference semantics):
    logits[h,l] = sum_d x[l,d] * w[h,l,d] / sqrt(D)
    attn = softmax(mask(logits), axis=l)
    v    = x @ v_w.T                      # (L,D)
    out  = attn @ v = (attn @ x) @ v_w.T  # algebraic refactor: 13x fewer MACs
    out  = LN(out); y = LN(FFN(out) + b2 + out)

Sharding: data-parallel over the 128 (b,s) pairs -> 16 per core x 8 cores.
All matmul inputs are bf16 (fp32 PSUM accumulation); LN/softmax math in fp32.

Key layouts on device (per core, 16 bs rows):
  xn  (l=128, bs=16, d=512)  natural      - lhsT for z^T = x^T(attn^T)
  xt  (dd=128, c=4, bs=16, l=128)         - rhs for the logits diag-matmul
  wl  (dd=128, g=8, c=4, (h,lp)=128)      - logits weights, l packed in groups of 16
  logits computed as out[(h,lp),(bs,l)] = sum_d wl.T xt per l-group, then the
  lp==l diagonal is extracted with a mask-multiply + segmented reduce.
"""
import sys

sys.path.insert(0, "/opt/trn_rl_repo")

import numpy as np

import concourse.bass as bass
import concourse.mybir as mybir
import concourse.tile as tile
from concourse.vector_clock import ScopedClock

B, S, H, L, D, DI = 4, 32, 8, 128, 512, 2048
EPS = 1e-6
TEMP = float(D) ** 0.5
N_CORES = 8
BSL = (B * S) // N_CORES  # 16 (b,s) rows per core
NEG = -1e9

F32 = mybir.dt.float32
BF16 = mybir.dt.bfloat16
FP8 = mybir.dt.float8e4
NP_BF16 = mybir.dt.np(BF16)
NP_FP8 = mybir.dt.np(FP8)


def _split_multi_waits(nc, max_waits=1):
    """walrus CoreV3 codegen rejects >1 sync-wait on an instruction; move
    extra waits onto injected same-engine NoOps placed just before."""
    for f in nc.m.functions:
        for bb in f.blocks:
            new = []
            for inst in bb.instructions:
                si = inst.sync_info
                if si is not None and len(si.on_wait) > max_waits:
                    waits = list(si.on_wait)
                    extra, keep = waits[:-max_waits], waits[-max_waits:]
                    for k, w in enumerate(extra):
                        new.append(
                            mybir.InstNoOp(
                                name=f"{inst.name}-wsplit{k}",
                                engine=inst.engine,
                                ins=[],
                                outs=[],
                                sync_info=mybir.SyncInfo(on_wait=[w], on_update=[]),
                            )
                        )
                    si.on_wait = keep
                new.append(inst)
            bb.instructions[:] = new


class _FastTailTC(tile.TileContext):
    """TileContext whose exit skips the two all-engine EVSEM barriers
    (~6-8us). The drain still waits on the global vector clock, so all
    work (incl. output DMAs) is complete; semaphore clears follow on the
    sync engine for the next execution of the same NEFF."""

    def _drain_and_barrier(self, tick_clock, wait_clock):
        # The sem clears below are emitted on GPSIMD, so the global-clock
        # waits must gate GPSIMD (not SP) or the clears race live engines.
        drain_inst = self.nc.gpsimd.drain()
        wait_clock.add_sem_waits(
            drain_inst.ins, ScopedClock({None: tick_clock.global_clock})
        )
        popped = self.nc._tile_sem_poison_stack.pop()
        assert popped is self._sem_poison
        assert self.sems is not None
        self.nc.clear_and_free_semaphores(list(self.sems.allocated().values()))


def _bcast_ap(dram_ap, parts):
    """DRAM (n,) vector -> AP replicated across `parts` partitions."""
    return bass.AP(
        tensor=dram_ap.tensor,
        offset=dram_ap.offset,
        ap=[[0, parts]] + list(dram_ap.ap),
    )


def _env_flag(name):
    import os
    return os.environ.get(name, "") not in ("", "0")


def _build_program():
    nc = bass.Bass("TRN2", num_devices=N_CORES)

    # ---- I/O ----
    d_xn = nc.declare_dram_parameter("xn", (L, BSL, D), BF16, isOutput=False)
    d_xt = nc.declare_dram_parameter("xt", (4, 128, BSL, L), FP8, isOutput=False)
    d_wl = nc.declare_dram_parameter("wl", (128, 8, 4, 128), FP8, isOutput=False)
    d_vwt = nc.declare_dram_parameter("vwt", (128, 4, D), BF16, isOutput=False)
    d_w1gt = nc.declare_dram_parameter("w1gt", (128, 4, DI), BF16, isOutput=False)
    d_w2t = nc.declare_dram_parameter("w2t", (128, 16, D), BF16, isOutput=False)
    d_b1p = nc.declare_dram_parameter("b1p", (128, 16), F32, isOutput=False)
    d_mneg = nc.declare_dram_parameter("mneg", (128, 8, BSL), F32, isOutput=False)
    d_vec4 = nc.declare_dram_parameter("vec4", (4 * D,), F32, isOutput=False)
    d_attn = nc.declare_dram_parameter("attn", (BSL, H, L), F32, isOutput=True)
    d_y = nc.declare_dram_parameter("y", (BSL * H, D), F32, isOutput=True)

    with (tile.TileContext if _env_flag('KNOTAIL') else _FastTailTC)(nc) as tc:
        with (
            tc.tile_pool(name="singles", bufs=1) as singles,
            tc.tile_pool(name="work", bufs=1) as work,
            tc.tile_pool(name="work2", bufs=2) as work2,
            tc.tile_pool(name="pg", bufs=4, space="PSUM") as pg_pool,
            tc.tile_pool(name="trz", bufs=2, space="PSUM") as trz_pool,
            tc.tile_pool(name="pacc", bufs=1, space="PSUM") as pacc_pool,
        ):
            # ---- on-chip constants (no DMA): iota -> eye + diag16 ----
            ii = singles.tile([128, 128], mybir.dt.int32)
            nc.gpsimd.iota(ii, pattern=[[1, 128]], base=0, channel_multiplier=-1)
            ieq = singles.tile([128, 128], mybir.dt.int32)
            nc.vector.tensor_scalar(
                ieq, ii, 0, None, mybir.AluOpType.is_equal
            )
            eyef = singles.tile([128, 128], F32)
            nc.vector.tensor_copy(eyef, ieq)
            eyeb = singles.tile([128, 128], BF16)
            nc.vector.tensor_copy(eyeb, ieq)
            # diag16[p, l] = (l == p % 16)  <=>  ((l - p) & 15) == 0
            i16a = singles.tile([128, 16], mybir.dt.int32)
            nc.vector.tensor_scalar(
                i16a, ii[:, 0:16], 15, None, mybir.AluOpType.bitwise_and
            )
            i16 = singles.tile([128, 16], mybir.dt.int32)
            nc.vector.tensor_scalar(
                i16, i16a, 0, None, mybir.AluOpType.is_equal
            )
            diag16 = singles.tile([128, 16], F32)
            nc.vector.tensor_copy(diag16, i16)
            eps_t = singles.tile([128, 1], F32)
            nc.vector.memset(eps_t, EPS)

            # ---- PE warmup: dense fp32 matmuls (4 cyc/row) trip the HAM
            # busy-window while input DMAs stream, so real matmuls run at
            # 2.4GHz from the start. No DMA dependency (memset operands).
            warm_l = singles.tile([128, 128], F32)
            nc.vector.memset(warm_l, 1.0)
            warm_r = singles.tile([128, D], F32)
            nc.vector.memset(warm_r, 1.0)
            for _ in range(3):
                pw = pacc_pool.tile([128, D], F32, tag="pacc")
                nc.tensor.matmul(pw, lhsT=warm_l, rhs=warm_r, start=True, stop=True)

            # ---- input loads, ordered by first consumer ----
            xt = singles.tile([128, 4, BSL, L], FP8)
            nc.sync.dma_start(out=xt[:, 0, :, :], in_=d_xt[0])
            wl = singles.tile([128, 8, 4, 128], FP8)
            for g in range(4):
                nc.sync.dma_start(out=wl[:, g, :, :], in_=d_wl[:, g])
            for c in range(1, 4):
                nc.sync.dma_start(out=xt[:, c, :, :], in_=d_xt[c])
            for g in range(4, 8):
                nc.sync.dma_start(out=wl[:, g, :, :], in_=d_wl[:, g])
            mnegT = singles.tile([128, 8, BSL], F32)
            nc.sync.dma_start(out=mnegT, in_=d_mneg[:])
            xn = singles.tile([L, BSL, D], BF16)
            nc.sync.dma_start(out=xn[0:64, :, :], in_=d_xn[0:64])
            nc.sync.dma_start(out=xn[64:128, :, :], in_=d_xn[64:128])
            vwt = singles.tile([128, 4, D], BF16)
            nc.sync.dma_start(out=vwt, in_=d_vwt[:])
            vec4 = singles.tile([128, 4, D], F32)
            nc.sync.dma_start(out=vec4[0:64], in_=_bcast_ap(d_vec4[:], 64))
            nc.sync.dma_start(out=vec4[64:128], in_=_bcast_ap(d_vec4[:], 64))
            b1p = singles.tile([128, 16], F32)
            nc.sync.dma_start(out=b1p, in_=d_b1p[:])
            w1gt = singles.tile([128, 4, DI], BF16)
            w2t = singles.tile([128, 16, D], BF16)
            for q in range(4):
                nc.sync.dma_start(
                    out=w1gt[:, :, q * 512 : (q + 1) * 512],
                    in_=d_w1gt[:, :, q * 512 : (q + 1) * 512],
                )
                if q >= 2:
                    nc.sync.dma_start(
                        out=w2t[:, (q - 2) * 4 : (q - 1) * 4, :],
                        in_=d_w2t[:, (q - 2) * 4 : (q - 1) * 4, :],
                    )
            for q in range(2, 4):
                nc.sync.dma_start(
                    out=w2t[:, q * 4 : (q + 1) * 4, :],
                    in_=d_w2t[:, q * 4 : (q + 1) * 4, :],
                )

            # ---- phase L: logits via diag-extraction matmuls, with mask
            # and exp fused right after the segmented reduce ----
            # Pexp (bs, h, g, lp) holds UNNORMALIZED exp(logits/TEMP) -
            # LN1 absorbs the softmax denominator (scale-invariance).
            Pexp = work.tile([BSL, H, 8, 16], F32)
            psgs = [
                pg_pool.tile([128, 2, BSL, 16], F32, name=f"psg{i}", tag="pg")
                for i in range(4)
            ]
            # Two accumulation groups share each PSUM bank, and a start=True
            # matmul clears has_written for the WHOLE bank - so pre-zero
            # with memset and never use start=True here.
            for psg in psgs:
                nc.vector.memset(psg, 0.0)
            for c in range(4):
                for gpi, psg in enumerate(psgs):
                    for gg in range(2):
                        g = gpi * 2 + gg
                        nc.tensor.matmul(
                            psg[:, gg, :, :],
                            lhsT=wl[:, g, c, :],
                            rhs=xt[:, c, :, g * 16 : (g + 1) * 16],
                            start=False,
                            stop=(c == 3),
                        )
            for gp, psg in enumerate(psgs):
                prod = work2.tile([128, 2, BSL, 16], F32, tag="prod")
                nc.vector.tensor_tensor(
                    prod, psg,
                    diag16[:, None, None, :].to_broadcast((128, 2, BSL, 16)),
                    mybir.AluOpType.mult,
                )
                tg2 = work2.tile([128, 2, BSL], F32, tag="tg")
                nc.vector.tensor_reduce(
                    tg2, prod, axis=mybir.AxisListType.X, op=mybir.AluOpType.add
                )
                tgm = work2.tile([128, 2, BSL], F32, tag="tgm")
                nc.vector.tensor_tensor(
                    tgm, tg2, mnegT[:, 2 * gp : 2 * gp + 2, :],
                    mybir.AluOpType.add,
                )
                te = work2.tile([128, 2, BSL], F32, tag="te")
                nc.scalar.activation(
                    out=te, in_=tgm,
                    func=mybir.ActivationFunctionType.Exp, scale=1.0 / TEMP,
                )
                for gg in range(2):
                    g = gp * 2 + gg
                    pst = trz_pool.tile([BSL, 128], F32, tag="trz")
                    nc.tensor.transpose(pst, te[:, gg, :], eyef)
                    if gg == 0:
                        nc.scalar.copy(Pexp[:, :, g, :], pst)
                    else:
                        nc.vector.tensor_copy(Pexp[:, :, g, :], pst)

            # HAM fillers: transposes don't register as PE-busy, so keep
            # the clock warm across the softmax/transpose window.
            for i in range(6):
                pf = pacc_pool.tile([128, 128], F32, name=f"pfa{i}", tag="pacc")
                nc.tensor.matmul(pf, lhsT=eyeb, rhs=eyeb, start=True, stop=True)

            # attn^T (l on partitions), unnormalized, for the z matmuls
            AT = work.tile([L, H, BSL], BF16)
            for h in range(H):
                pat = trz_pool.tile([L, BSL], F32, tag="trz")
                nc.tensor.transpose(pat, Pexp[:, h, :, :], eyef[:BSL, :BSL])
                nc.scalar.copy(AT[:, h, :], pat)

            for i in range(4):
                pf = pacc_pool.tile([128, 128], F32, name=f"pfz{i}", tag="pacc")
                nc.tensor.matmul(pf, lhsT=eyeb, rhs=eyeb, start=True, stop=True)

            # ---- phase Z: zraw^T[d,(bs,h)] = sum_l x[l,d] expT[l,h] ----
            zT = work.tile([128, 4, BSL, H], BF16)
            for c in range(4):
                psz = pg_pool.tile([128, BSL, H], F32, tag="pg")
                for b in range(BSL):
                    nc.tensor.matmul(
                        psz[:, b, :],
                        lhsT=xn[:, b, c * 128 : (c + 1) * 128],
                        rhs=AT[:, :, b],
                        start=True,
                        stop=True,
                    )
                nc.vector.tensor_copy(zT[:, c, :, :], psz)

            # attn output (normalized) - off the critical path
            Ssum = work.tile([BSL, H], F32)
            nc.vector.tensor_reduce(
                Ssum, Pexp, axis=mybir.AxisListType.XY, op=mybir.AluOpType.add
            )
            Rsum = work.tile([BSL, H], F32)
            nc.vector.reciprocal(Rsum, Ssum)
            ATTN = work.tile([BSL, H, 8, 16], F32)
            for h in range(H):
                nc.vector.tensor_scalar_mul(
                    ATTN[:, h, :, :], Pexp[:, h, :, :], Rsum[:, h : h + 1]
                )
            nc.sync.dma_start(out=d_attn[:], in_=ATTN)

            g_bc = vec4[:, 0, :]
            b2_bc = vec4[:, 1, :]
            fg_bc = vec4[:, 2, :]
            fb_bc = vec4[:, 3, :]

            # ---- phase O: out2[(bs,h),e] = sum_d zT[d,(bs,h)] v_w[e,d] ----
            pso = pacc_pool.tile([128, D], F32, tag="pacc")
            for c in range(4):
                nc.tensor.matmul(
                    pso,
                    lhsT=zT[:, c, :, :],
                    rhs=vwt[:, c, :],
                    start=(c == 0),
                    stop=(c == 3),
                )

            # ---- LN1 (scale-invariant: absorbs the softmax 1/S) ----
            st6 = work.tile([128, 6], F32, tag="st6")
            nc.vector.bn_stats(st6, pso)
            mv = work.tile([128, 2], F32, tag="mv")
            nc.vector.bn_aggr(mv, st6)
            sd = work.tile([128, 1], F32, tag="sd")
            nc.scalar.activation(
                out=sd, in_=mv[:, 1:2], func=mybir.ActivationFunctionType.Sqrt,
                bias=eps_t[:, 0:1], scale=1.0,
            )
            rsig = work.tile([128, 1], F32, tag="rsig")
            nc.vector.reciprocal(rsig, sd)
            t1 = work.tile([128, D], F32, tag="t1")
            nc.vector.tensor_scalar(
                t1, pso, mv[:, 0:1], rsig,
                mybir.AluOpType.subtract, mybir.AluOpType.mult,
            )

            # ---- FFN (ln_g is folded into w1gt; ln_b into b1p/b2pp) ----
            T4 = work.tile([128, 4, 128], BF16)  # t1^T chunks (dd, bsh)
            for c in range(4):
                pstc = trz_pool.tile([128, 128], F32, tag="trz")
                nc.tensor.transpose(pstc, t1[:, c * 128 : (c + 1) * 128], eyef)
                nc.scalar.copy(T4[:, c, :], pstc)

            # residual = t1*ln_g + (ln_b + b2), off the critical path (POOL)
            tg1 = work.tile([128, D], F32, tag="tgl")
            nc.gpsimd.tensor_tensor(tg1, t1, g_bc, mybir.AluOpType.mult)
            resid = work.tile([128, D], F32, tag="resid")
            nc.gpsimd.tensor_tensor(resid, tg1, b2_bc, mybir.AluOpType.add)

            for i in range(4):
                pf = pacc_pool.tile([128, 128], F32, name=f"pff{i}", tag="pacc")
                nc.tensor.matmul(pf, lhsT=eyeb, rhs=eyeb, start=True, stop=True)

            h1 = work.tile([128, 16, 128], BF16)  # relu(h1^T) (ff, fc, bsh)
            for fc in range(16):
                psh = pg_pool.tile([128, 128], F32, tag="pg")
                for c in range(4):
                    nc.tensor.matmul(
                        psh,
                        lhsT=w1gt[:, c, fc * 128 : (fc + 1) * 128],
                        rhs=T4[:, c, :],
                        start=(c == 0),
                        stop=(c == 3),
                    )
                nc.scalar.activation(
                    out=h1[:, fc, :], in_=psh,
                    func=mybir.ActivationFunctionType.Relu,
                    bias=b1p[:, fc : fc + 1], scale=1.0,
                )

            psy = pacc_pool.tile([128, D], F32, tag="pacc")
            for fc in range(16):
                nc.tensor.matmul(
                    psy,
                    lhsT=h1[:, fc, :],
                    rhs=w2t[:, fc, :],
                    start=(fc == 0),
                    stop=(fc == 15),
                )
            ypre = work.tile([128, D], F32, tag="ypre")
            nc.vector.tensor_tensor(ypre, psy, resid, mybir.AluOpType.add)

            # ---- LN2 + affine ----
            st6b = work.tile([128, 6], F32, tag="st6b")
            nc.vector.bn_stats(st6b, ypre)
            mv2 = work.tile([128, 2], F32, tag="mv2")
            nc.vector.bn_aggr(mv2, st6b)
            sd2 = work.tile([128, 1], F32, tag="sd2")
            nc.scalar.activation(
                out=sd2, in_=mv2[:, 1:2], func=mybir.ActivationFunctionType.Sqrt,
                bias=eps_t[:, 0:1], scale=1.0,
            )
            rsig2 = work.tile([128, 1], F32, tag="rsig2")
            nc.vector.reciprocal(rsig2, sd2)
            t2 = work.tile([128, D], F32, tag="t2")
            nc.vector.tensor_scalar(
                t2, ypre, mv2[:, 0:1], rsig2,
                mybir.AluOpType.subtract, mybir.AluOpType.mult,
            )
            yg = work.tile([128, D], F32, tag="yg")
            yout = work.tile([128, D], F32, tag="yout")
            HD = D // 2
            nc.vector.tensor_tensor(
                yg[:, :HD], t2[:, :HD], fg_bc[:, :HD], mybir.AluOpType.mult
            )
            nc.vector.tensor_tensor(
                yout[:, :HD], yg[:, :HD], fb_bc[:, :HD], mybir.AluOpType.add
            )
            nc.sync.dma_start(out=d_y[:, :HD], in_=yout[:, :HD])
            nc.gpsimd.tensor_tensor(
                yg[:, HD:], t2[:, HD:], fg_bc[:, HD:], mybir.AluOpType.mult
            )
            nc.gpsimd.tensor_tensor(
                yout[:, HD:], yg[:, HD:], fb_bc[:, HD:], mybir.AluOpType.add
            )
            nc.sync.dma_start(out=d_y[:, HD:], in_=yout[:, HD:])

    _split_multi_waits(nc)
    return nc


def _host_prep(x, mask, w, v_w, ln_g, ln_b, w1, b1, w2, b2, fln_g, fln_b):
    """Build the per-core input maps (all static packing done on host)."""
    x = np.asarray(x, np.float32).reshape(B * S, L, D)
    mask = np.asarray(mask, np.int32).reshape(B * S, L)
    w0 = np.asarray(w, np.float32).reshape(H, L, D)
    v_w = np.asarray(v_w, np.float32)
    ln_g = np.asarray(ln_g, np.float32)
    ln_b = np.asarray(ln_b, np.float32)
    w1 = np.asarray(w1, np.float32)
    b1 = np.asarray(b1, np.float32)
    w2 = np.asarray(w2, np.float32)
    b2 = np.asarray(b2, np.float32)
    fln_g = np.asarray(fln_g, np.float32)
    fln_b = np.asarray(fln_b, np.float32)

    # shared (replicated) tensors
    # wl[dd, g, c, (h,lp)] = w0[h, g*16+lp, c*128+dd]
    wl = np.ascontiguousarray(
        w0.reshape(H, 8, 16, 4, 128).transpose(4, 1, 3, 0, 2).reshape(128, 8, 4, 128)
    ).astype(NP_FP8)
    vwt = np.ascontiguousarray(
        v_w.T.reshape(4, 128, D).transpose(1, 0, 2)
    ).astype(NP_BF16)
    w1g = w1 * ln_g[None, :]
    w1gt = np.ascontiguousarray(
        w1g.T.reshape(4, 128, DI).transpose(1, 0, 2)
    ).astype(NP_BF16)
    w2t = np.ascontiguousarray(
        w2.T.reshape(16, 128, D).transpose(1, 0, 2)
    ).astype(NP_BF16)
    b1p = np.ascontiguousarray((b1 + w1 @ ln_b).reshape(16, 128).T).astype(np.float32)
    b2pp = (b2 + ln_b).astype(np.float32)
    # diagm[h*16+lp, bs, l] = (lp == l): build as (h, lp, bs, l)

    vec4 = np.concatenate([ln_g, b2pp, fln_g, fln_b]).astype(np.float32)
    shared = {
        "wl": wl, "vwt": vwt, "w1gt": w1gt, "w2t": w2t, "b1p": b1p,
        "vec4": vec4,
    }

    in_maps = []
    for ci in range(N_CORES):
        lo, hi = ci * BSL, (ci + 1) * BSL
        xs = x[lo:hi]  # (16, L, D) f32
        xn = np.ascontiguousarray(xs.transpose(1, 0, 2)).astype(NP_BF16)
        xt = np.ascontiguousarray(
            xs.reshape(BSL, L, 4, 128).transpose(2, 3, 0, 1)
        ).astype(NP_FP8)
        mneg1 = np.where(mask[lo:hi] == 0, np.float32(NEG), np.float32(0.0))
        # mnegT[(h,lp), g, bs] = mneg1[bs, g*16+lp]
        mneg = np.ascontiguousarray(
            np.broadcast_to(
                mneg1.reshape(BSL, 8, 16).transpose(2, 1, 0)[None], (H, 16, 8, BSL)
            ).reshape(128, 8, BSL)
        )
        in_maps.append({"xn": xn, "xt": xt, "mneg": mneg, **shared})
    return in_maps


_CACHE = {}


def _get_runner():
    """Compile once; return a callable(in_maps) -> list[dict] per core."""
    if "runner" in _CACHE:
        return _CACHE["runner"]

    import jax
    from jax.sharding import Mesh, PartitionSpec
    from jax.experimental.shard_map import shard_map

    from concourse import bass2jax

    bass2jax.install_neuronx_cc_hook()
    nc = _build_program()

    in_names, out_names, out_avals, zero_outs = [], [], [], []
    for alloc in nc.m.functions[0].allocations:
        if not isinstance(alloc, mybir.MemoryLocationSet):
            continue
        name = alloc.memorylocations[0].name
        if alloc.kind == "ExternalInput":
            if nc.partition_id_tensor is None or name != nc.partition_id_tensor.name:
                in_names.append(name)
        elif alloc.kind == "ExternalOutput":
            shape = tuple(alloc.tensor_shape)
            dtype = mybir.dt.np(alloc.dtype)
            out_names.append(name)
            out_avals.append(jax.core.ShapedArray(shape, dtype))
            zero_outs.append(np.zeros(shape, dtype))
    n_params = len(in_names)
    all_names = in_names + out_names
    if nc.partition_id_tensor is not None:
        all_names.append(nc.partition_id_tensor.name)

    def _body(*args):
        operands = list(args)
        if nc.partition_id_tensor is not None:
            operands.append(bass2jax.partition_id_tensor())
        outs = bass2jax._bass_exec_p.bind(
            *operands,
            out_avals=tuple(out_avals),
            in_names=tuple(all_names),
            out_names=tuple(out_names),
            lowering_input_output_aliases=(),
            sim_require_finite=True,
            sim_require_nnan=True,
            nc=nc,
        )
        return tuple(outs)

    devices = jax.devices()[:N_CORES]
    mesh = Mesh(np.asarray(devices), ("core",))
    nin = n_params + len(out_names)
    sharded = jax.jit(
        shard_map(
            _body,
            mesh=mesh,
            in_specs=(PartitionSpec("core"),) * nin,
            out_specs=(PartitionSpec("core"),) * len(out_names),
            check_rep=False,
        ),
        keep_unused=True,
    )

    concat_zeros = [
        np.zeros((N_CORES * z.shape[0], *z.shape[1:]), z.dtype) for z in zero_outs
    ]

    def run(in_maps, repeats=1, time_it=False):
        import time as _time

        concat_in = [
            np.concatenate([np.asarray(m[name]) for m in in_maps], axis=0)
            for name in in_names
        ]
        args = [jax.device_put(a) for a in concat_in + concat_zeros]
        out = sharded(*args)
        jax.block_until_ready(out)
        dt = None
        if time_it:
            t0 = _time.perf_counter()
            for _ in range(repeats):
                out = sharded(*args)
            jax.block_until_ready(out)
            dt = (_time.perf_counter() - t0) / repeats
        results = [
            {
                name: np.asarray(out[i]).reshape(N_CORES, *out_avals[i].shape)[c]
                for i, name in enumerate(out_names)
            }
            for c in range(N_CORES)
        ]
        return results, dt

    _CACHE["runner"] = run
    return run


def kernel(**inputs):
    run = _get_runner()
    in_maps = _host_prep(**inputs)
    results, _ = run(in_maps)
    y = np.concatenate(
        [r["y"].reshape(BSL, H, D) for r in results], axis=0
    ).reshape(B, S, H, D).astype(np.float32)
    attn = np.concatenate(
        [r["attn"] for r in results], axis=0
    ).reshape(B, S, H, L).astype(np.float32)
    return y, attn


def kernel_timed(repeats=20, **inputs):
    """For test.py: returns (y, attn), per-iteration seconds."""
    run = _get_runner()
    in_maps = _host_prep(**inputs)
    results, dt = run(in_maps, repeats=repeats, time_it=True)
    y = np.concatenate(
        [r["y"].reshape(BSL, H, D) for r in results], axis=0
    ).reshape(B, S, H, D).astype(np.float32)
    attn = np.concatenate(
        [r["attn"] for r in results], axis=0
    ).reshape(B, S, H, L).astype(np.float32)
    return (y, attn), dt


# revision 32
# speedup vs baseline: 1.1226x; 1.1226x over previous
"""Trainium2 Bass kernel for nn_MultiHeadReadOutAttention.

Computation (per (b,s), reference semantics):
    logits[h,l] = sum_d x[l,d] * w[h,l,d] / sqrt(D)
    attn = softmax(mask(logits), axis=l)
    v    = x @ v_w.T                      # (L,D)
    out  = attn @ v = (attn @ x) @ v_w.T  # algebraic refactor: 13x fewer MACs
    out  = LN(out); y = LN(FFN(out) + b2 + out)

Sharding: data-parallel over the 128 (b,s) pairs -> 16 per core x 8 cores.
All matmul inputs are bf16 (fp32 PSUM accumulation); LN/softmax math in fp32.

Key layouts on device (per core, 16 bs rows):
  xn  (l=128, bs=16, d=512)  natural      - lhsT for z^T = x^T(attn^T)
  xt  (dd=128, c=4, bs=16, l=128)         - rhs for the logits diag-matmul
  wl  (dd=128, g=8, c=4, (h,lp)=128)      - logits weights, l packed in groups of 16
  logits computed as out[(h,lp),(bs,l)] = sum_d wl.T xt per l-group, then the
  lp==l diagonal is extracted with a mask-multiply + segmented reduce.
"""
import sys

sys.path.insert(0, "/opt/trn_rl_repo")

import numpy as np

import concourse.bass as bass
import concourse.mybir as mybir
import concourse.tile as tile
from concourse.vector_clock import ScopedClock

B, S, H, L, D, DI = 4, 32, 8, 128, 512, 2048
EPS = 1e-6
TEMP = float(D) ** 0.5
N_CORES = 8
BSL = (B * S) // N_CORES  # 16 (b,s) rows per core
NEG = -1e9

F32 = mybir.dt.float32
BF16 = mybir.dt.bfloat16
FP8 = mybir.dt.float8e4
NP_BF16 = mybir.dt.np(BF16)
NP_FP8 = mybir.dt.np(FP8)


def _split_multi_waits(nc, max_waits=1):
    """walrus CoreV3 codegen rejects >1 sync-wait on an instruction; move
    extra waits onto injected same-engine NoOps placed just before."""
    for f in nc.m.functions:
        for bb in f.blocks:
            new = []
            for inst in bb.instructions:
                si = inst.sync_info
                if si is not None and len(si.on_wait) > max_waits:
                    waits = list(si.on_wait)
                    extra, keep = waits[:-max_waits], waits[-max_waits:]
                    for k, w in enumerate(extra):
                        new.append(
                            mybir.InstNoOp(
                                name=f"{inst.name}-wsplit{k}",
                                engine=inst.engine,
                                ins=[],
                                outs=[],
                                sync_info=mybir.SyncInfo(on_wait=[w], on_update=[]),
                            )
                        )
                    si.on_wait = keep
                new.append(inst)
            bb.instructions[:] = new


class _FastTailTC(tile.TileContext):
    """TileContext whose exit skips the two all-engine EVSEM barriers
    (~6-8us). The drain still waits on the global vector clock, so all
    work (incl. output DMAs) is complete; semaphore clears follow on the
    sync engine for the next execution of the same NEFF."""

    def _drain_and_barrier(self, tick_clock, wait_clock):
        # The sem clears below are emitted on GPSIMD, so the global-clock
        # waits must gate GPSIMD (not SP) or the clears race live engines.
        drain_inst = self.nc.gpsimd.drain()
        wait_clock.add_sem_waits(
            drain_inst.ins, ScopedClock({None: tick_clock.global_clock})
        )
        popped = self.nc._tile_sem_poison_stack.pop()
        assert popped is self._sem_poison
        assert self.sems is not None
        self.nc.clear_and_free_semaphores(list(self.sems.allocated().values()))


def _bcast_ap(dram_ap, parts):
    """DRAM (n,) vector -> AP replicated across `parts` partitions."""
    return bass.AP(
        tensor=dram_ap.tensor,
        offset=dram_ap.offset,
        ap=[[0, parts]] + list(dram_ap.ap),
    )


def _env_flag(name):
    import os
    return os.environ.get(name, "") not in ("", "0")


def _build_program():
    nc = bass.Bass("TRN2", num_devices=N_CORES)

    # ---- I/O ----
    d_xn = nc.declare_dram_parameter("xn", (L, BSL, D), BF16, isOutput=False)
    d_xt = nc.declare_dram_parameter("xt", (4, 128, BSL, L), FP8, isOutput=False)
    d_wl = nc.declare_dram_parameter("wl", (128, 8, 4, 128), FP8, isOutput=False)
    d_vwt = nc.declare_dram_parameter("vwt", (128, 4, D), BF16, isOutput=False)
    d_w1gt = nc.declare_dram_parameter("w1gt", (128, 4, DI), BF16, isOutput=False)
    d_w2t = nc.declare_dram_parameter("w2t", (128, 16, D), BF16, isOutput=False)
    d_b1p = nc.declare_dram_parameter("b1p", (128, 16), F32, isOutput=False)
    d_mneg = nc.declare_dram_parameter("mneg", (128, 8, BSL), F32, isOutput=False)
    d_vec4 = nc.declare_dram_parameter("vec4", (4 * D,), F32, isOutput=False)
    d_attn = nc.declare_dram_parameter("attn", (BSL, H, L), F32, isOutput=True)
    d_y = nc.declare_dram_parameter("y", (BSL * H, D), F32, isOutput=True)

    with (tile.TileContext if _env_flag('KNOTAIL') else _FastTailTC)(nc) as tc:
        with (
            tc.tile_pool(name="singles", bufs=1) as singles,
            tc.tile_pool(name="work", bufs=1) as work,
            tc.tile_pool(name="work2", bufs=2) as work2,
            tc.tile_pool(name="pg", bufs=4, space="PSUM") as pg_pool,
            tc.tile_pool(name="trz", bufs=2, space="PSUM") as trz_pool,
            tc.tile_pool(name="pacc", bufs=1, space="PSUM") as pacc_pool,
        ):
            # ---- on-chip constants (no DMA): iota -> eye + diag16 ----
            ii = singles.tile([128, 128], mybir.dt.int32)
            nc.gpsimd.iota(ii, pattern=[[1, 128]], base=0, channel_multiplier=-1)
            ieq = singles.tile([128, 128], mybir.dt.int32)
            nc.vector.tensor_scalar(
                ieq, ii, 0, None, mybir.AluOpType.is_equal
            )
            eyef = singles.tile([128, 128], F32)
            nc.vector.tensor_copy(eyef, ieq)
            eyeb = singles.tile([128, 128], BF16)
            nc.vector.tensor_copy(eyeb, ieq)
            # diag16[p, l] = (l == p % 16)  <=>  ((l - p) & 15) == 0
            i16a = singles.tile([128, 16], mybir.dt.int32)
            nc.vector.tensor_scalar(
                i16a, ii[:, 0:16], 15, None, mybir.AluOpType.bitwise_and
            )
            i16 = singles.tile([128, 16], mybir.dt.int32)
            nc.vector.tensor_scalar(
                i16, i16a, 0, None, mybir.AluOpType.is_equal
            )
            diag16 = singles.tile([128, 16], F32)
            nc.vector.tensor_copy(diag16, i16)
            eps_t = singles.tile([128, 1], F32)
            nc.vector.memset(eps_t, EPS)

            # ---- PE warmup: dense fp32 matmuls (4 cyc/row) trip the HAM
            # busy-window while input DMAs stream, so real matmuls run at
            # 2.4GHz from the start. No DMA dependency (memset operands).
            warm_l = singles.tile([128, 128], F32)
            nc.vector.memset(warm_l, 1.0)
            warm_r = singles.tile([128, D], F32)
            nc.vector.memset(warm_r, 1.0)
            for _ in range(3):
                pw = pacc_pool.tile([128, D], F32, tag="pacc")
                nc.tensor.matmul(pw, lhsT=warm_l, rhs=warm_r, start=True, stop=True)

            # ---- input loads, ordered by first consumer ----
            xt = singles.tile([128, 4, BSL, L], FP8)
            nc.sync.dma_start(out=xt[:, 0, :, :], in_=d_xt[0])
            wl = singles.tile([128, 8, 4, 128], FP8)
            for g in range(4):
                nc.sync.dma_start(out=wl[:, g, :, :], in_=d_wl[:, g])
            for c in range(1, 4):
                nc.sync.dma_start(out=xt[:, c, :, :], in_=d_xt[c])
            for g in range(4, 8):
                nc.sync.dma_start(out=wl[:, g, :, :], in_=d_wl[:, g])
            mnegT = singles.tile([128, 8, BSL], F32)
            nc.sync.dma_start(out=mnegT, in_=d_mneg[:])
            xn = singles.tile([L, BSL, D], BF16)
            nc.sync.dma_start(out=xn, in_=d_xn[:])
            vwt = singles.tile([128, 4, D], BF16)
            nc.sync.dma_start(out=vwt, in_=d_vwt[:])
            vec4 = singles.tile([128, 4, D], F32)
            nc.sync.dma_start(out=vec4, in_=_bcast_ap(d_vec4[:], 128))
            b1p = singles.tile([128, 16], F32)
            nc.sync.dma_start(out=b1p, in_=d_b1p[:])
            w1gt = singles.tile([128, 4, DI], BF16)
            w2t = singles.tile([128, 16, D], BF16)
            for q in range(4):
                nc.sync.dma_start(
                    out=w1gt[:, :, q * 512 : (q + 1) * 512],
                    in_=d_w1gt[:, :, q * 512 : (q + 1) * 512],
                )
                if q >= 2:
                    nc.sync.dma_start(
                        out=w2t[:, (q - 2) * 4 : (q - 1) * 4, :],
                        in_=d_w2t[:, (q - 2) * 4 : (q - 1) * 4, :],
                    )
            for q in range(2, 4):
                nc.sync.dma_start(
                    out=w2t[:, q * 4 : (q + 1) * 4, :],
                    in_=d_w2t[:, q * 4 : (q + 1) * 4, :],
                )

            # ---- phase L: logits via diag-extraction matmuls, with mask
            # and exp fused right after the segmented reduce ----
            # Pexp (bs, h, g, lp) holds UNNORMALIZED exp(logits/TEMP) -
            # LN1 absorbs the softmax denominator (scale-invariance).
            Pexp = work.tile([BSL, H, 8, 16], F32)
            psgs = [
                pg_pool.tile([128, 2, BSL, 16], F32, name=f"psg{i}", tag="pg")
                for i in range(4)
            ]
            # Two accumulation groups share each PSUM bank, and a start=True
            # matmul clears has_written for the WHOLE bank - so pre-zero
            # with memset and never use start=True here.
            for psg in psgs:
                nc.vector.memset(psg, 0.0)
            for c in range(4):
                for gpi, psg in enumerate(psgs):
                    for gg in range(2):
                        g = gpi * 2 + gg
                        nc.tensor.matmul(
                            psg[:, gg, :, :],
                            lhsT=wl[:, g, c, :],
                            rhs=xt[:, c, :, g * 16 : (g + 1) * 16],
                            start=False,
                            stop=(c == 3),
                        )
            for gp, psg in enumerate(psgs):
                prod = work2.tile([128, 2, BSL, 16], F32, tag="prod")
                nc.vector.tensor_tensor(
                    prod, psg,
                    diag16[:, None, None, :].to_broadcast((128, 2, BSL, 16)),
                    mybir.AluOpType.mult,
                )
                tg2 = work2.tile([128, 2, BSL], F32, tag="tg")
                nc.vector.tensor_reduce(
                    tg2, prod, axis=mybir.AxisListType.X, op=mybir.AluOpType.add
                )
                tgm = work2.tile([128, 2, BSL], F32, tag="tgm")
                nc.vector.tensor_tensor(
                    tgm, tg2, mnegT[:, 2 * gp : 2 * gp + 2, :],
                    mybir.AluOpType.add,
                )
                te = work2.tile([128, 2, BSL], F32, tag="te")
                nc.scalar.activation(
                    out=te, in_=tgm,
                    func=mybir.ActivationFunctionType.Exp, scale=1.0 / TEMP,
                )
                for gg in range(2):
                    g = gp * 2 + gg
                    pst = trz_pool.tile([BSL, 128], F32, tag="trz")
                    nc.tensor.transpose(pst, te[:, gg, :], eyef)
                    if gg == 0:
                        nc.scalar.copy(Pexp[:, :, g, :], pst)
                    else:
                        nc.vector.tensor_copy(Pexp[:, :, g, :], pst)

            # HAM fillers: transposes don't register as PE-busy, so keep
            # the clock warm across the softmax/transpose window.
            for i in range(6):
                pf = pacc_pool.tile([128, 128], F32, name=f"pfa{i}", tag="pacc")
                nc.tensor.matmul(pf, lhsT=eyeb, rhs=eyeb, start=True, stop=True)

            # attn^T (l on partitions), unnormalized, for the z matmuls
            AT = work.tile([L, H, BSL], BF16)
            for h in range(H):
                pat = trz_pool.tile([L, BSL], F32, tag="trz")
                nc.tensor.transpose(pat, Pexp[:, h, :, :], eyef[:BSL, :BSL])
                nc.scalar.copy(AT[:, h, :], pat)

            for i in range(4):
                pf = pacc_pool.tile([128, 128], F32, name=f"pfz{i}", tag="pacc")
                nc.tensor.matmul(pf, lhsT=eyeb, rhs=eyeb, start=True, stop=True)

            # ---- phase Z: zraw^T[d,(bs,h)] = sum_l x[l,d] expT[l,h] ----
            zT = work.tile([128, 4, BSL, H], BF16)
            for c in range(4):
                psz = pg_pool.tile([128, BSL, H], F32, tag="pg")
                for b in range(BSL):
                    nc.tensor.matmul(
                        psz[:, b, :],
                        lhsT=xn[:, b, c * 128 : (c + 1) * 128],
                        rhs=AT[:, :, b],
                        start=True,
                        stop=True,
                    )
                nc.vector.tensor_copy(zT[:, c, :, :], psz)

            # attn output (normalized) - off the critical path
            Ssum = work.tile([BSL, H], F32)
            nc.vector.tensor_reduce(
                Ssum, Pexp, axis=mybir.AxisListType.XY, op=mybir.AluOpType.add
            )
            Rsum = work.tile([BSL, H], F32)
            nc.vector.reciprocal(Rsum, Ssum)
            ATTN = work.tile([BSL, H, 8, 16], F32)
            for h in range(H):
                nc.vector.tensor_scalar_mul(
                    ATTN[:, h, :, :], Pexp[:, h, :, :], Rsum[:, h : h + 1]
                )
            nc.sync.dma_start(out=d_attn[:], in_=ATTN)

            g_bc = vec4[:, 0, :]
            b2_bc = vec4[:, 1, :]
            fg_bc = vec4[:, 2, :]
            fb_bc = vec4[:, 3, :]

            # ---- phase O: out2[(bs,h),e] = sum_d zT[d,(bs,h)] v_w[e,d] ----
            pso = pacc_pool.tile([128, D], F32, tag="pacc")
            for c in range(4):
                nc.tensor.matmul(
                    pso,
                    lhsT=zT[:, c, :, :],
                    rhs=vwt[:, c, :],
                    start=(c == 0),
                    stop=(c == 3),
                )

            # ---- LN1 (scale-invariant: absorbs the softmax 1/S) ----
            st6 = work.tile([128, 6], F32, tag="st6")
            nc.vector.bn_stats(st6, pso)
            mv = work.tile([128, 2], F32, tag="mv")
            nc.vector.bn_aggr(mv, st6)
            sd = work.tile([128, 1], F32, tag="sd")
            nc.scalar.activation(
                out=sd, in_=mv[:, 1:2], func=mybir.ActivationFunctionType.Sqrt,
                bias=eps_t[:, 0:1], scale=1.0,
            )
            rsig = work.tile([128, 1], F32, tag="rsig")
            nc.vector.reciprocal(rsig, sd)
            t1 = work.tile([128, D], F32, tag="t1")
            nc.vector.tensor_scalar(
                t1, pso, mv[:, 0:1], rsig,
                mybir.AluOpType.subtract, mybir.AluOpType.mult,
            )

            # ---- FFN (ln_g is folded into w1gt; ln_b into b1p/b2pp) ----
            T4 = work.tile([128, 4, 128], BF16)  # t1^T chunks (dd, bsh)
            for c in range(4):
                pstc = trz_pool.tile([128, 128], F32, tag="trz")
                nc.tensor.transpose(pstc, t1[:, c * 128 : (c + 1) * 128], eyef)
                nc.scalar.copy(T4[:, c, :], pstc)

            # residual = t1*ln_g + (ln_b + b2), off the critical path (POOL)
            tg1 = work.tile([128, D], F32, tag="tgl")
            nc.gpsimd.tensor_tensor(tg1, t1, g_bc, mybir.AluOpType.mult)
            resid = work.tile([128, D], F32, tag="resid")
            nc.gpsimd.tensor_tensor(resid, tg1, b2_bc, mybir.AluOpType.add)

            for i in range(4):
                pf = pacc_pool.tile([128, 128], F32, name=f"pff{i}", tag="pacc")
                nc.tensor.matmul(pf, lhsT=eyeb, rhs=eyeb, start=True, stop=True)

            h1 = work.tile([128, 16, 128], BF16)  # relu(h1^T) (ff, fc, bsh)
            for fc in range(16):
                psh = pg_pool.tile([128, 128], F32, tag="pg")
                for c in range(4):
                    nc.tensor.matmul(
                        psh,
                        lhsT=w1gt[:, c, fc * 128 : (fc + 1) * 128],
                        rhs=T4[:, c, :],
                        start=(c == 0),
                        stop=(c == 3),
                    )
                nc.scalar.activation(
                    out=h1[:, fc, :], in_=psh,
                    func=mybir.ActivationFunctionType.Relu,
                    bias=b1p[:, fc : fc + 1], scale=1.0,
                )

            psy = pacc_pool.tile([128, D], F32, tag="pacc")
            for fc in range(16):
                nc.tensor.matmul(
                    psy,
                    lhsT=h1[:, fc, :],
                    rhs=w2t[:, fc, :],
                    start=(fc == 0),
                    stop=(fc == 15),
                )
            ypre = work.tile([128, D], F32, tag="ypre")
            nc.vector.tensor_tensor(ypre, psy, resid, mybir.AluOpType.add)

            # ---- LN2 + affine ----
            st6b = work.tile([128, 6], F32, tag="st6b")
            nc.vector.bn_stats(st6b, ypre)
            mv2 = work.tile([128, 2], F32, tag="mv2")
            nc.vector.bn_aggr(mv2, st6b)
            sd2 = work.tile([128, 1], F32, tag="sd2")
            nc.scalar.activation(
                out=sd2, in_=mv2[:, 1:2], func=mybir.ActivationFunctionType.Sqrt,
                bias=eps_t[:, 0:1], scale=1.0,
            )
            rsig2 = work.tile([128, 1], F32, tag="rsig2")
            nc.vector.reciprocal(rsig2, sd2)
            t2 = work.tile([128, D], F32, tag="t2")
            nc.vector.tensor_scalar(
                t2, ypre, mv2[:, 0:1], rsig2,
                mybir.AluOpType.subtract, mybir.AluOpType.mult,
            )
            yg = work.tile([128, D], F32, tag="yg")
            nc.vector.tensor_tensor(yg, t2, fg_bc, mybir.AluOpType.mult)
            yout = work.tile([128, D], F32, tag="yout")
            nc.vector.tensor_tensor(yout, yg, fb_bc, mybir.AluOpType.add)
            nc.sync.dma_start(out=d_y[:], in_=yout)

    _split_multi_waits(nc)
    return nc


def _host_prep(x, mask, w, v_w, ln_g, ln_b, w1, b1, w2, b2, fln_g, fln_b):
    """Build the per-core input maps (all static packing done on host)."""
    x = np.asarray(x, np.float32).reshape(B * S, L, D)
    mask = np.asarray(mask, np.int32).reshape(B * S, L)
    w0 = np.asarray(w, np.float32).reshape(H, L, D)
    v_w = np.asarray(v_w, np.float32)
    ln_g = np.asarray(ln_g, np.float32)
    ln_b = np.asarray(ln_b, np.float32)
    w1 = np.asarray(w1, np.float32)
    b1 = np.asarray(b1, np.float32)
    w2 = np.asarray(w2, np.float32)
    b2 = np.asarray(b2, np.float32)
    fln_g = np.asarray(fln_g, np.float32)
    fln_b = np.asarray(fln_b, np.float32)

    # shared (replicated) tensors
    # wl[dd, g, c, (h,lp)] = w0[h, g*16+lp, c*128+dd]
    wl = np.ascontiguousarray(
        w0.reshape(H, 8, 16, 4, 128).transpose(4, 1, 3, 0, 2).reshape(128, 8, 4, 128)
    ).astype(NP_FP8)
    vwt = np.ascontiguousarray(
        v_w.T.reshape(4, 128, D).transpose(1, 0, 2)
    ).astype(NP_BF16)
    w1g = w1 * ln_g[None, :]
    w1gt = np.ascontiguousarray(
        w1g.T.reshape(4, 128, DI).transpose(1, 0, 2)
    ).astype(NP_BF16)
    w2t = np.ascontiguousarray(
        w2.T.reshape(16, 128, D).transpose(1, 0, 2)
    ).astype(NP_BF16)
    b1p = np.ascontiguousarray((b1 + w1 @ ln_b).reshape(16, 128).T).astype(np.float32)
    b2pp = (b2 + ln_b).astype(np.float32)
    # diagm[h*16+lp, bs, l] = (lp == l): build as (h, lp, bs, l)

    vec4 = np.concatenate([ln_g, b2pp, fln_g, fln_b]).astype(np.float32)
    shared = {
        "wl": wl, "vwt": vwt, "w1gt": w1gt, "w2t": w2t, "b1p": b1p,
        "vec4": vec4,
    }

    in_maps = []
    for ci in range(N_CORES):
        lo, hi = ci * BSL, (ci + 1) * BSL
        xs = x[lo:hi]  # (16, L, D) f32
        xn = np.ascontiguousarray(xs.transpose(1, 0, 2)).astype(NP_BF16)
        xt = np.ascontiguousarray(
            xs.reshape(BSL, L, 4, 128).transpose(2, 3, 0, 1)
        ).astype(NP_FP8)
        mneg1 = np.where(mask[lo:hi] == 0, np.float32(NEG), np.float32(0.0))
        # mnegT[(h,lp), g, bs] = mneg1[bs, g*16+lp]
        mneg = np.ascontiguousarray(
            np.broadcast_to(
                mneg1.reshape(BSL, 8, 16).transpose(2, 1, 0)[None], (H, 16, 8, BSL)
            ).reshape(128, 8, BSL)
        )
        in_maps.append({"xn": xn, "xt": xt, "mneg": mneg, **shared})
    return in_maps


_CACHE = {}


def _get_runner():
    """Compile once; return a callable(in_maps) -> list[dict] per core."""
    if "runner" in _CACHE:
        return _CACHE["runner"]

    import jax
    from jax.sharding import Mesh, PartitionSpec
    from jax.experimental.shard_map import shard_map

    from concourse import bass2jax

    bass2jax.install_neuronx_cc_hook()
    nc = _build_program()

    in_names, out_names, out_avals, zero_outs = [], [], [], []
    for alloc in nc.m.functions[0].allocations:
        if not isinstance(alloc, mybir.MemoryLocationSet):
            continue
        name = alloc.memorylocations[0].name
        if alloc.kind == "ExternalInput":
            if nc.partition_id_tensor is None or name != nc.partition_id_tensor.name:
                in_names.append(name)
        elif alloc.kind == "ExternalOutput":
            shape = tuple(alloc.tensor_shape)
            dtype = mybir.dt.np(alloc.dtype)
            out_names.append(name)
            out_avals.append(jax.core.ShapedArray(shape, dtype))
            zero_outs.append(np.zeros(shape, dtype))
    n_params = len(in_names)
    all_names = in_names + out_names
    if nc.partition_id_tensor is not None:
        all_names.append(nc.partition_id_tensor.name)

    def _body(*args):
        operands = list(args)
        if nc.partition_id_tensor is not None:
            operands.append(bass2jax.partition_id_tensor())
        outs = bass2jax._bass_exec_p.bind(
            *operands,
            out_avals=tuple(out_avals),
            in_names=tuple(all_names),
            out_names=tuple(out_names),
            lowering_input_output_aliases=(),
            sim_require_finite=True,
            sim_require_nnan=True,
            nc=nc,
        )
        return tuple(outs)

    devices = jax.devices()[:N_CORES]
    mesh = Mesh(np.asarray(devices), ("core",))
    nin = n_params + len(out_names)
    sharded = jax.jit(
        shard_map(
            _body,
            mesh=mesh,
            in_specs=(PartitionSpec("core"),) * nin,
            out_specs=(PartitionSpec("core"),) * len(out_names),
            check_rep=False,
        ),
        keep_unused=True,
    )

    concat_zeros = [
        np.zeros((N_CORES * z.shape[0], *z.shape[1:]), z.dtype) for z in zero_outs
    ]

    def run(in_maps, repeats=1, time_it=False):
        import time as _time

        concat_in = [
            np.concatenate([np.asarray(m[name]) for m in in_maps], axis=0)
            for name in in_names
        ]
        args = [jax.device_put(a) for a in concat_in + concat_zeros]
        out = sharded(*args)
        jax.block_until_ready(out)
        dt = None
        if time_it:
            t0 = _time.perf_counter()
            for _ in range(repeats):
                out = sharded(*args)
            jax.block_until_ready(out)
            dt = (_time.perf_counter() - t0) / repeats
        results = [
            {
                name: np.asarray(out[i]).reshape(N_CORES, *out_avals[i].shape)[c]
                for i, name in enumerate(out_names)
            }
            for c in range(N_CORES)
        ]
        return results, dt

    _CACHE["runner"] = run
    return run


def kernel(**inputs):
    run = _get_runner()
    in_maps = _host_prep(**inputs)
    results, _ = run(in_maps)
    y = np.concatenate(
        [r["y"].reshape(BSL, H, D) for r in results], axis=0
    ).reshape(B, S, H, D).astype(np.float32)
    attn = np.concatenate(
        [r["attn"] for r in results], axis=0
    ).reshape(B, S, H, L).astype(np.float32)
    return y, attn


def kernel_timed(repeats=20, **inputs):
    """For test.py: returns (y, attn), per-iteration seconds."""
    run = _get_runner()
    in_maps = _host_prep(**inputs)
    results, dt = run(in_maps, repeats=repeats, time_it=True)
    y = np.concatenate(
        [r["y"].reshape(BSL, H, D) for r in results], axis=0
    ).reshape(B, S, H, D).astype(np.float32)
    attn = np.concatenate(
        [r["attn"] for r in results], axis=0
    ).reshape(B, S, H, L).astype(np.float32)
    return (y, attn), dt


# revision 33
# speedup vs baseline: 1.1598x; 1.0332x over previous
"""Trainium2 Bass kernel for nn_MultiHeadReadOutAttention.

Computation (per (b,s), reference semantics):
    logits[h,l] = sum_d x[l,d] * w[h,l,d] / sqrt(D)
    attn = softmax(mask(logits), axis=l)
    v    = x @ v_w.T                      # (L,D)
    out  = attn @ v = (attn @ x) @ v_w.T  # algebraic refactor: 13x fewer MACs
    out  = LN(out); y = LN(FFN(out) + b2 + out)

Sharding: data-parallel over the 128 (b,s) pairs -> 16 per core x 8 cores.
All matmul inputs are bf16 (fp32 PSUM accumulation); LN/softmax math in fp32.

Key layouts on device (per core, 16 bs rows):
  xn  (l=128, bs=16, d=512)  natural      - lhsT for z^T = x^T(attn^T)
  xt  (dd=128, c=4, bs=16, l=128)         - rhs for the logits diag-matmul
  wl  (dd=128, g=8, c=4, (h,lp)=128)      - logits weights, l packed in groups of 16
  logits computed as out[(h,lp),(bs,l)] = sum_d wl.T xt per l-group, then the
  lp==l diagonal is extracted with a mask-multiply + segmented reduce.
"""
import sys

sys.path.insert(0, "/opt/trn_rl_repo")

import numpy as np

import concourse.bass as bass
import concourse.mybir as mybir
import concourse.tile as tile
from concourse.vector_clock import ScopedClock

B, S, H, L, D, DI = 4, 32, 8, 128, 512, 2048
EPS = 1e-6
TEMP = float(D) ** 0.5
N_CORES = 8
BSL = (B * S) // N_CORES  # 16 (b,s) rows per core
NEG = -1e9

F32 = mybir.dt.float32
BF16 = mybir.dt.bfloat16
FP8 = mybir.dt.float8e4
NP_BF16 = mybir.dt.np(BF16)
NP_FP8 = mybir.dt.np(FP8)


def _split_multi_waits(nc, max_waits=1):
    """walrus CoreV3 codegen rejects >1 sync-wait on an instruction; move
    extra waits onto injected same-engine NoOps placed just before."""
    for f in nc.m.functions:
        for bb in f.blocks:
            new = []
            for inst in bb.instructions:
                si = inst.sync_info
                if si is not None and len(si.on_wait) > max_waits:
                    waits = list(si.on_wait)
                    extra, keep = waits[:-max_waits], waits[-max_waits:]
                    for k, w in enumerate(extra):
                        new.append(
                            mybir.InstNoOp(
                                name=f"{inst.name}-wsplit{k}",
                                engine=inst.engine,
                                ins=[],
                                outs=[],
                                sync_info=mybir.SyncInfo(on_wait=[w], on_update=[]),
                            )
                        )
                    si.on_wait = keep
                new.append(inst)
            bb.instructions[:] = new


class _FastTailTC(tile.TileContext):
    """TileContext whose exit skips the two all-engine EVSEM barriers
    (~6-8us). The drain still waits on the global vector clock, so all
    work (incl. output DMAs) is complete; semaphore clears follow on the
    sync engine for the next execution of the same NEFF."""

    def _drain_and_barrier(self, tick_clock, wait_clock):
        # The sem clears below are emitted on GPSIMD, so the global-clock
        # waits must gate GPSIMD (not SP) or the clears race live engines.
        drain_inst = self.nc.gpsimd.drain()
        wait_clock.add_sem_waits(
            drain_inst.ins, ScopedClock({None: tick_clock.global_clock})
        )
        popped = self.nc._tile_sem_poison_stack.pop()
        assert popped is self._sem_poison
        assert self.sems is not None
        self.nc.clear_and_free_semaphores(list(self.sems.allocated().values()))


def _bcast_ap(dram_ap, parts):
    """DRAM (n,) vector -> AP replicated across `parts` partitions."""
    return bass.AP(
        tensor=dram_ap.tensor,
        offset=dram_ap.offset,
        ap=[[0, parts]] + list(dram_ap.ap),
    )


def _env_flag(name):
    import os
    return os.environ.get(name, "") not in ("", "0")


def _build_program():
    nc = bass.Bass("TRN2", num_devices=N_CORES)

    # ---- I/O ----
    d_xn = nc.declare_dram_parameter("xn", (L, BSL, D), BF16, isOutput=False)
    d_xt = nc.declare_dram_parameter("xt", (4, 128, BSL, L), FP8, isOutput=False)
    d_wl = nc.declare_dram_parameter("wl", (128, 8, 4, 128), FP8, isOutput=False)
    d_vwt = nc.declare_dram_parameter("vwt", (128, 4, D), BF16, isOutput=False)
    d_w1gt = nc.declare_dram_parameter("w1gt", (128, 4, DI), BF16, isOutput=False)
    d_w2t = nc.declare_dram_parameter("w2t", (128, 16, D), BF16, isOutput=False)
    d_b1p = nc.declare_dram_parameter("b1p", (128, 16), F32, isOutput=False)
    d_mneg = nc.declare_dram_parameter("mneg", (128, 8, BSL), F32, isOutput=False)
    d_vec4 = nc.declare_dram_parameter("vec4", (4 * D,), F32, isOutput=False)
    d_attn = nc.declare_dram_parameter("attn", (BSL, H, L), F32, isOutput=True)
    d_y = nc.declare_dram_parameter("y", (BSL * H, D), F32, isOutput=True)

    with (tile.TileContext if _env_flag('KNOTAIL') else _FastTailTC)(nc) as tc:
        with (
            tc.tile_pool(name="singles", bufs=1) as singles,
            tc.tile_pool(name="work", bufs=1) as work,
            tc.tile_pool(name="work2", bufs=3) as work2,
            tc.tile_pool(name="pg", bufs=4, space="PSUM") as pg_pool,
            tc.tile_pool(name="trz", bufs=3, space="PSUM") as trz_pool,
            tc.tile_pool(name="pacc", bufs=1, space="PSUM") as pacc_pool,
        ):
            # ---- on-chip constants (no DMA): iota -> eye + diag16 ----
            ii = singles.tile([128, 128], mybir.dt.int32)
            nc.gpsimd.iota(ii, pattern=[[1, 128]], base=0, channel_multiplier=-1)
            ieq = singles.tile([128, 128], mybir.dt.int32)
            nc.vector.tensor_scalar(
                ieq, ii, 0, None, mybir.AluOpType.is_equal
            )
            eyef = singles.tile([128, 128], F32)
            nc.vector.tensor_copy(eyef, ieq)
            eyeb = singles.tile([128, 128], BF16)
            nc.vector.tensor_copy(eyeb, ieq)
            # diag16[p, l] = (l == p % 16)  <=>  ((l - p) & 15) == 0
            i16a = singles.tile([128, 16], mybir.dt.int32)
            nc.vector.tensor_scalar(
                i16a, ii[:, 0:16], 15, None, mybir.AluOpType.bitwise_and
            )
            i16 = singles.tile([128, 16], mybir.dt.int32)
            nc.vector.tensor_scalar(
                i16, i16a, 0, None, mybir.AluOpType.is_equal
            )
            diag16 = singles.tile([128, 16], F32)
            nc.vector.tensor_copy(diag16, i16)
            eps_t = singles.tile([128, 1], F32)
            nc.vector.memset(eps_t, EPS)

            # ---- PE warmup: dense fp32 matmuls (4 cyc/row) trip the HAM
            # busy-window while input DMAs stream, so real matmuls run at
            # 2.4GHz from the start. No DMA dependency (memset operands).
            warm_l = singles.tile([128, 128], F32)
            nc.vector.memset(warm_l, 1.0)
            warm_r = singles.tile([128, D], F32)
            nc.vector.memset(warm_r, 1.0)
            for _ in range(3):
                pw = pacc_pool.tile([128, D], F32, tag="pacc")
                nc.tensor.matmul(pw, lhsT=warm_l, rhs=warm_r, start=True, stop=True)

            # ---- input loads, ordered by first consumer ----
            xt = singles.tile([128, 4, BSL, L], FP8)
            nc.sync.dma_start(out=xt[:, 0, :, :], in_=d_xt[0])
            wl = singles.tile([128, 8, 4, 128], FP8)
            for g in range(4):
                nc.sync.dma_start(out=wl[:, g, :, :], in_=d_wl[:, g])
            for c in range(1, 4):
                nc.sync.dma_start(out=xt[:, c, :, :], in_=d_xt[c])
            for g in range(4, 8):
                nc.sync.dma_start(out=wl[:, g, :, :], in_=d_wl[:, g])
            mnegT = singles.tile([128, 8, BSL], F32)
            nc.sync.dma_start(out=mnegT, in_=d_mneg[:])
            xn = singles.tile([L, BSL, D], BF16)
            nc.sync.dma_start(out=xn, in_=d_xn[:])
            vwt = singles.tile([128, 4, D], BF16)
            nc.sync.dma_start(out=vwt, in_=d_vwt[:])
            vec4 = singles.tile([128, 4, D], F32)
            nc.sync.dma_start(out=vec4, in_=_bcast_ap(d_vec4[:], 128))
            b1p = singles.tile([128, 16], F32)
            nc.sync.dma_start(out=b1p, in_=d_b1p[:])
            w1gt = singles.tile([128, 4, DI], BF16)
            w2t = singles.tile([128, 16, D], BF16)
            for q in range(4):
                nc.sync.dma_start(
                    out=w1gt[:, :, q * 512 : (q + 1) * 512],
                    in_=d_w1gt[:, :, q * 512 : (q + 1) * 512],
                )
                if q >= 2:
                    nc.sync.dma_start(
                        out=w2t[:, (q - 2) * 4 : (q - 1) * 4, :],
                        in_=d_w2t[:, (q - 2) * 4 : (q - 1) * 4, :],
                    )
            for q in range(2, 4):
                nc.sync.dma_start(
                    out=w2t[:, q * 4 : (q + 1) * 4, :],
                    in_=d_w2t[:, q * 4 : (q + 1) * 4, :],
                )

            # ---- phase L: logits via diag-extraction matmuls, with mask
            # and exp fused right after the segmented reduce ----
            # Pexp (bs, h, g, lp) holds UNNORMALIZED exp(logits/TEMP) -
            # LN1 absorbs the softmax denominator (scale-invariance).
            Pexp = work.tile([BSL, H, 8, 16], F32)
            psgs = [
                pg_pool.tile([128, 2, BSL, 16], F32, name=f"psg{i}", tag="pg")
                for i in range(4)
            ]
            # Two accumulation groups share each PSUM bank, and a start=True
            # matmul clears has_written for the WHOLE bank - so pre-zero
            # with memset and never use start=True here.
            for psg in psgs:
                nc.vector.memset(psg, 0.0)
            for c in range(4):
                for gpi, psg in enumerate(psgs):
                    for gg in range(2):
                        g = gpi * 2 + gg
                        nc.tensor.matmul(
                            psg[:, gg, :, :],
                            lhsT=wl[:, g, c, :],
                            rhs=xt[:, c, :, g * 16 : (g + 1) * 16],
                            start=False,
                            stop=(c == 3),
                        )
            for gp, psg in enumerate(psgs):
                prod = work2.tile([128, 2, BSL, 16], F32, tag="prod")
                nc.vector.tensor_tensor(
                    prod, psg,
                    diag16[:, None, None, :].to_broadcast((128, 2, BSL, 16)),
                    mybir.AluOpType.mult,
                )
                tg2 = work2.tile([128, 2, BSL], F32, tag="tg")
                nc.vector.tensor_reduce(
                    tg2, prod, axis=mybir.AxisListType.X, op=mybir.AluOpType.add
                )
                tgm = work2.tile([128, 2, BSL], F32, tag="tgm")
                nc.vector.tensor_tensor(
                    tgm, tg2, mnegT[:, 2 * gp : 2 * gp + 2, :],
                    mybir.AluOpType.add,
                )
                te = work2.tile([128, 2, BSL], F32, tag="te")
                nc.scalar.activation(
                    out=te, in_=tgm,
                    func=mybir.ActivationFunctionType.Exp, scale=1.0 / TEMP,
                )
                for gg in range(2):
                    g = gp * 2 + gg
                    pst = trz_pool.tile([BSL, 128], F32, tag="trz")
                    nc.tensor.transpose(pst, te[:, gg, :], eyef)
                    if gg == 0:
                        nc.scalar.copy(Pexp[:, :, g, :], pst)
                    else:
                        nc.vector.tensor_copy(Pexp[:, :, g, :], pst)

            # HAM fillers: transposes don't register as PE-busy, so keep
            # the clock warm across the softmax/transpose window.
            for i in range(6):
                pf = pacc_pool.tile([128, 128], F32, name=f"pfa{i}", tag="pacc")
                nc.tensor.matmul(pf, lhsT=eyeb, rhs=eyeb, start=True, stop=True)

            # attn^T (l on partitions), unnormalized, for the z matmuls
            AT = work.tile([L, H, BSL], BF16)
            for h in range(H):
                pat = trz_pool.tile([L, BSL], F32, tag="trz")
                nc.tensor.transpose(pat, Pexp[:, h, :, :], eyef[:BSL, :BSL])
                nc.scalar.copy(AT[:, h, :], pat)

            for i in range(4):
                pf = pacc_pool.tile([128, 128], F32, name=f"pfz{i}", tag="pacc")
                nc.tensor.matmul(pf, lhsT=eyeb, rhs=eyeb, start=True, stop=True)

            # ---- phase Z: zraw^T[d,(bs,h)] = sum_l x[l,d] expT[l,h] ----
            zT = work.tile([128, 4, BSL, H], BF16)
            for c in range(4):
                psz = pg_pool.tile([128, BSL, H], F32, tag="pg")
                for b in range(BSL):
                    nc.tensor.matmul(
                        psz[:, b, :],
                        lhsT=xn[:, b, c * 128 : (c + 1) * 128],
                        rhs=AT[:, :, b],
                        start=True,
                        stop=True,
                    )
                nc.vector.tensor_copy(zT[:, c, :, :], psz)

            # attn output (normalized) - off the critical path
            Ssum = work.tile([BSL, H], F32)
            nc.vector.tensor_reduce(
                Ssum, Pexp, axis=mybir.AxisListType.XY, op=mybir.AluOpType.add
            )
            Rsum = work.tile([BSL, H], F32)
            nc.vector.reciprocal(Rsum, Ssum)
            ATTN = work.tile([BSL, H, 8, 16], F32)
            for h in range(H):
                nc.vector.tensor_scalar_mul(
                    ATTN[:, h, :, :], Pexp[:, h, :, :], Rsum[:, h : h + 1]
                )
            nc.sync.dma_start(out=d_attn[:], in_=ATTN)

            g_bc = vec4[:, 0, :]
            b2_bc = vec4[:, 1, :]
            fg_bc = vec4[:, 2, :]
            fb_bc = vec4[:, 3, :]

            # ---- phase O: out2[(bs,h),e] = sum_d zT[d,(bs,h)] v_w[e,d] ----
            pso = pacc_pool.tile([128, D], F32, tag="pacc")
            for c in range(4):
                nc.tensor.matmul(
                    pso,
                    lhsT=zT[:, c, :, :],
                    rhs=vwt[:, c, :],
                    start=(c == 0),
                    stop=(c == 3),
                )

            # ---- LN1 (scale-invariant: absorbs the softmax 1/S) ----
            st6 = work.tile([128, 6], F32, tag="st6")
            nc.vector.bn_stats(st6, pso)
            mv = work.tile([128, 2], F32, tag="mv")
            nc.vector.bn_aggr(mv, st6)
            sd = work.tile([128, 1], F32, tag="sd")
            nc.scalar.activation(
                out=sd, in_=mv[:, 1:2], func=mybir.ActivationFunctionType.Sqrt,
                bias=eps_t[:, 0:1], scale=1.0,
            )
            rsig = work.tile([128, 1], F32, tag="rsig")
            nc.vector.reciprocal(rsig, sd)
            t1 = work.tile([128, D], F32, tag="t1")
            nc.vector.tensor_scalar(
                t1, pso, mv[:, 0:1], rsig,
                mybir.AluOpType.subtract, mybir.AluOpType.mult,
            )

            # ---- FFN (ln_g is folded into w1gt; ln_b into b1p/b2pp) ----
            T4 = work.tile([128, 4, 128], BF16)  # t1^T chunks (dd, bsh)
            for c in range(4):
                pstc = trz_pool.tile([128, 128], F32, tag="trz")
                nc.tensor.transpose(pstc, t1[:, c * 128 : (c + 1) * 128], eyef)
                nc.scalar.copy(T4[:, c, :], pstc)

            # residual = t1*ln_g + (ln_b + b2), off the critical path (POOL)
            tg1 = work.tile([128, D], F32, tag="tgl")
            nc.gpsimd.tensor_tensor(tg1, t1, g_bc, mybir.AluOpType.mult)
            resid = work.tile([128, D], F32, tag="resid")
            nc.gpsimd.tensor_tensor(resid, tg1, b2_bc, mybir.AluOpType.add)

            for i in range(4):
                pf = pacc_pool.tile([128, 128], F32, name=f"pff{i}", tag="pacc")
                nc.tensor.matmul(pf, lhsT=eyeb, rhs=eyeb, start=True, stop=True)

            h1 = work.tile([128, 16, 128], BF16)  # relu(h1^T) (ff, fc, bsh)
            for fc in range(16):
                psh = pg_pool.tile([128, 128], F32, tag="pg")
                for c in range(4):
                    nc.tensor.matmul(
                        psh,
                        lhsT=w1gt[:, c, fc * 128 : (fc + 1) * 128],
                        rhs=T4[:, c, :],
                        start=(c == 0),
                        stop=(c == 3),
                    )
                nc.scalar.activation(
                    out=h1[:, fc, :], in_=psh,
                    func=mybir.ActivationFunctionType.Relu,
                    bias=b1p[:, fc : fc + 1], scale=1.0,
                )

            psy = pacc_pool.tile([128, D], F32, tag="pacc")
            for fc in range(16):
                nc.tensor.matmul(
                    psy,
                    lhsT=h1[:, fc, :],
                    rhs=w2t[:, fc, :],
                    start=(fc == 0),
                    stop=(fc == 15),
                )
            ypre = work.tile([128, D], F32, tag="ypre")
            nc.vector.tensor_tensor(ypre, psy, resid, mybir.AluOpType.add)

            # ---- LN2 + affine ----
            st6b = work.tile([128, 6], F32, tag="st6b")
            nc.vector.bn_stats(st6b, ypre)
            mv2 = work.tile([128, 2], F32, tag="mv2")
            nc.vector.bn_aggr(mv2, st6b)
            sd2 = work.tile([128, 1], F32, tag="sd2")
            nc.scalar.activation(
                out=sd2, in_=mv2[:, 1:2], func=mybir.ActivationFunctionType.Sqrt,
                bias=eps_t[:, 0:1], scale=1.0,
            )
            rsig2 = work.tile([128, 1], F32, tag="rsig2")
            nc.vector.reciprocal(rsig2, sd2)
            t2 = work.tile([128, D], F32, tag="t2")
            nc.vector.tensor_scalar(
                t2, ypre, mv2[:, 0:1], rsig2,
                mybir.AluOpType.subtract, mybir.AluOpType.mult,
            )
            yg = work.tile([128, D], F32, tag="yg")
            nc.vector.tensor_tensor(yg, t2, fg_bc, mybir.AluOpType.mult)
            yout = work.tile([128, D], F32, tag="yout")
            nc.vector.tensor_tensor(yout, yg, fb_bc, mybir.AluOpType.add)
            nc.sync.dma_start(out=d_y[:], in_=yout)

    _split_multi_waits(nc)
    return nc


def _host_prep(x, mask, w, v_w, ln_g, ln_b, w1, b1, w2, b2, fln_g, fln_b):
    """Build the per-core input maps (all static packing done on host)."""
    x = np.asarray(x, np.float32).reshape(B * S, L, D)
    mask = np.asarray(mask, np.int32).reshape(B * S, L)
    w0 = np.asarray(w, np.float32).reshape(H, L, D)
    v_w = np.asarray(v_w, np.float32)
    ln_g = np.asarray(ln_g, np.float32)
    ln_b = np.asarray(ln_b, np.float32)
    w1 = np.asarray(w1, np.float32)
    b1 = np.asarray(b1, np.float32)
    w2 = np.asarray(w2, np.float32)
    b2 = np.asarray(b2, np.float32)
    fln_g = np.asarray(fln_g, np.float32)
    fln_b = np.asarray(fln_b, np.float32)

    # shared (replicated) tensors
    # wl[dd, g, c, (h,lp)] = w0[h, g*16+lp, c*128+dd]
    wl = np.ascontiguousarray(
        w0.reshape(H, 8, 16, 4, 128).transpose(4, 1, 3, 0, 2).reshape(128, 8, 4, 128)
    ).astype(NP_FP8)
    vwt = np.ascontiguousarray(
        v_w.T.reshape(4, 128, D).transpose(1, 0, 2)
    ).astype(NP_BF16)
    w1g = w1 * ln_g[None, :]
    w1gt = np.ascontiguousarray(
        w1g.T.reshape(4, 128, DI).transpose(1, 0, 2)
    ).astype(NP_BF16)
    w2t = np.ascontiguousarray(
        w2.T.reshape(16, 128, D).transpose(1, 0, 2)
    ).astype(NP_BF16)
    b1p = np.ascontiguousarray((b1 + w1 @ ln_b).reshape(16, 128).T).astype(np.float32)
    b2pp = (b2 + ln_b).astype(np.float32)
    # diagm[h*16+lp, bs, l] = (lp == l): build as (h, lp, bs, l)

    vec4 = np.concatenate([ln_g, b2pp, fln_g, fln_b]).astype(np.float32)
    shared = {
        "wl": wl, "vwt": vwt, "w1gt": w1gt, "w2t": w2t, "b1p": b1p,
        "vec4": vec4,
    }

    in_maps = []
    for ci in range(N_CORES):
        lo, hi = ci * BSL, (ci + 1) * BSL
        xs = x[lo:hi]  # (16, L, D) f32
        xn = np.ascontiguousarray(xs.transpose(1, 0, 2)).astype(NP_BF16)
        xt = np.ascontiguousarray(
            xs.reshape(BSL, L, 4, 128).transpose(2, 3, 0, 1)
        ).astype(NP_FP8)
        mneg1 = np.where(mask[lo:hi] == 0, np.float32(NEG), np.float32(0.0))
        # mnegT[(h,lp), g, bs] = mneg1[bs, g*16+lp]
        mneg = np.ascontiguousarray(
            np.broadcast_to(
                mneg1.reshape(BSL, 8, 16).transpose(2, 1, 0)[None], (H, 16, 8, BSL)
            ).reshape(128, 8, BSL)
        )
        in_maps.append({"xn": xn, "xt": xt, "mneg": mneg, **shared})
    return in_maps


_CACHE = {}


def _get_runner():
    """Compile once; return a callable(in_maps) -> list[dict] per core."""
    if "runner" in _CACHE:
        return _CACHE["runner"]

    import jax
    from jax.sharding import Mesh, PartitionSpec
    from jax.experimental.shard_map import shard_map

    from concourse import bass2jax

    bass2jax.install_neuronx_cc_hook()
    nc = _build_program()

    in_names, out_names, out_avals, zero_outs = [], [], [], []
    for alloc in nc.m.functions[0].allocations:
        if not isinstance(alloc, mybir.MemoryLocationSet):
            continue
        name = alloc.memorylocations[0].name
        if alloc.kind == "ExternalInput":
            if nc.partition_id_tensor is None or name != nc.partition_id_tensor.name:
                in_names.append(name)
        elif alloc.kind == "ExternalOutput":
            shape = tuple(alloc.tensor_shape)
            dtype = mybir.dt.np(alloc.dtype)
            out_names.append(name)
            out_avals.append(jax.core.ShapedArray(shape, dtype))
            zero_outs.append(np.zeros(shape, dtype))
    n_params = len(in_names)
    all_names = in_names + out_names
    if nc.partition_id_tensor is not None:
        all_names.append(nc.partition_id_tensor.name)

    def _body(*args):
        operands = list(args)
        if nc.partition_id_tensor is not None:
            operands.append(bass2jax.partition_id_tensor())
        outs = bass2jax._bass_exec_p.bind(
            *operands,
            out_avals=tuple(out_avals),
            in_names=tuple(all_names),
            out_names=tuple(out_names),
            lowering_input_output_aliases=(),
            sim_require_finite=True,
            sim_require_nnan=True,
            nc=nc,
        )
        return tuple(outs)

    devices = jax.devices()[:N_CORES]
    mesh = Mesh(np.asarray(devices), ("core",))
    nin = n_params + len(out_names)
    sharded = jax.jit(
        shard_map(
            _body,
            mesh=mesh,
            in_specs=(PartitionSpec("core"),) * nin,
            out_specs=(PartitionSpec("core"),) * len(out_names),
            check_rep=False,
        ),
        keep_unused=True,
    )

    concat_zeros = [
        np.zeros((N_CORES * z.shape[0], *z.shape[1:]), z.dtype) for z in zero_outs
    ]

    def run(in_maps, repeats=1, time_it=False):
        import time as _time

        concat_in = [
            np.concatenate([np.asarray(m[name]) for m in in_maps], axis=0)
            for name in in_names
        ]
        args = [jax.device_put(a) for a in concat_in + concat_zeros]
        out = sharded(*args)
        jax.block_until_ready(out)
        dt = None
        if time_it:
            t0 = _time.perf_counter()
            for _ in range(repeats):
                out = sharded(*args)
            jax.block_until_ready(out)
            dt = (_time.perf_counter() - t0) / repeats
        results = [
            {
                name: np.asarray(out[i]).reshape(N_CORES, *out_avals[i].shape)[c]
                for i, name in enumerate(out_names)
            }
            for c in range(N_CORES)
        ]
        return results, dt

    _CACHE["runner"] = run
    return run


def kernel(**inputs):
    run = _get_runner()
    in_maps = _host_prep(**inputs)
    results, _ = run(in_maps)
    y = np.concatenate(
        [r["y"].reshape(BSL, H, D) for r in results], axis=0
    ).reshape(B, S, H, D).astype(np.float32)
    attn = np.concatenate(
        [r["attn"] for r in results], axis=0
    ).reshape(B, S, H, L).astype(np.float32)
    return y, attn


def kernel_timed(repeats=20, **inputs):
    """For test.py: returns (y, attn), per-iteration seconds."""
    run = _get_runner()
    in_maps = _host_prep(**inputs)
    results, dt = run(in_maps, repeats=repeats, time_it=True)
    y = np.concatenate(
        [r["y"].reshape(BSL, H, D) for r in results], axis=0
    ).reshape(B, S, H, D).astype(np.float32)
    attn = np.concatenate(
        [r["attn"] for r in results], axis=0
    ).reshape(B, S, H, L).astype(np.float32)
    return (y, attn), dt
